# revision 12
# baseline (speedup 1.0000x reference)
"""Trainium2 Bass kernel for nn_InteractionBlock (gnn_message_passing).

Algebraic transformation: per angle alpha with (s, t) = (src, tgt):
    sm[alpha] = (msg[s] @ Ws + bs) * d[t]
    out[alpha] = sum_b a[t, b] * (Wb[:, b, :] @ sm[alpha])
    agg[t] = sum_{alpha: tgt=t} out[alpha]
Everything except msg[s] depends only on t, so with
    S[t] = sum_{alpha: tgt=t} msg[s(alpha)]   and  c[t] = |{alpha: tgt=t}|
    agg[t] = sum_b a[t,b] * (Wb[:,b,:] @ ((S[t] @ Ws + c[t]*bs) * d[t]))
The A=600K gather+einsum+scatter collapses to an E-sized dense pipeline.
S (a sparse-matrix product) and a = x_angle @ Wa ([E, 8]) are cheap on the
host. The wall-clock budget is dominated by the axon host->device link, so
everything is tuned for bytes-on-the-wire: bulk tensors travel as bf16,
operands are packed into 6 arrays (vs ~29), host->device copies are issued
asynchronously before the XLA/NEFF compile so they overlap, and no
zero-initialized donation buffers are shipped (the kernel writes every
element of the output). The device runs only the dense per-edge pipeline,
edges sharded 8 ways, feature-major.
"""

import os
import sys

import numpy as np

sys.path.insert(0, "/opt/trn_rl_repo")

E = 100000
NR = 6
NS = 7
H = 128
BD = 8
M = 128
P = 8           # cores
ES = E // P     # 12500 edges per core
NT = 512        # dense-phase column tile
NSP = 12800     # padded edges per core (25 * 512)
NTILES = NSP // NT  # 25

# packed bf16 weight slab: name -> (col offset, rows, cols)
_WOFF = {}
_c = 0
for _n, _r, _k in [("Wd", NR, H), ("Ws", M, H), ("bs_row", 1, H),
                   ("Wt", M, H), ("bt_row", 1, H), ("WbT", H, BD * H),
                   ("rb_w1", H, H), ("rb_w2", H, H), ("Wskip", H, M),
                   ("ra1_w1", M, M), ("ra1_w2", M, M),
                   ("ra2_w1", M, M), ("ra2_w2", M, M)]:
    _WOFF[_n] = (_c, _r, _k)
    _c += _k
WCOLS = _c  # 2560
_BIAS_NAMES = ["rb_b1", "rb_b2", "bskip", "ra1_b1", "ra1_b2",
               "ra2_b1", "ra2_b2"]


def _bf16():
    import ml_dtypes
    return np.dtype(ml_dtypes.bfloat16)


def _pack_weights(w):
    """All matmul weights into one [128, WCOLS] bf16 slab; biases into
    [128, 7] f32."""
    bf16 = _bf16()
    WB = np.zeros((128, WCOLS), bf16)
    for name, (c0, rows, cols) in _WOFF.items():
        WB[:rows, c0:c0 + cols] = w[name]
    BIA = np.zeros((128, len(_BIAS_NAMES)), np.float32)
    for i, name in enumerate(_BIAS_NAMES):
        BIA[:, i] = w[name]
    return WB, BIA


def _build(nc, tc, aps):
    """Dense per-edge pipeline, feature-major, bf16 in / f32 accumulate."""
    from contextlib import ExitStack

    from concourse import mybir

    f32 = mybir.dt.float32
    bf16 = mybir.dt.bfloat16
    Silu = mybir.ActivationFunctionType.Silu
    mult = mybir.AluOpType.mult

    with ExitStack() as ctx:
        wpool = ctx.enter_context(tc.tile_pool(name="w", bufs=1))

        wslab = wpool.tile([128, WCOLS], bf16, tag="WB")
        nc.sync.dma_start(wslab[:], aps["WB"][:])
        bias = wpool.tile([128, len(_BIAS_NAMES)], f32, tag="BIA")
        nc.sync.dma_start(bias[:], aps["BIA"][:])

        def W(name):
            c0, rows, cols = _WOFF[name]
            return wslab[0:rows, c0:c0 + cols]

        def B(name):
            return bias[:, _BIAS_NAMES.index(name):_BIAS_NAMES.index(name) + 1]

        ones_row = wpool.tile([1, NT], bf16, tag="ones")
        nc.gpsimd.memset(ones_row[:], 1.0)
        ones_col = wpool.tile([1, 128], bf16, tag="onesc")
        nc.gpsimd.memset(ones_col[:], 1.0)

        dense = ctx.enter_context(tc.tile_pool(name="dn", bufs=3))
        pacc = ctx.enter_context(tc.tile_pool(name="pacc", bufs=2,
                                              space="PSUM"))
        psc = ctx.enter_context(tc.tile_pool(name="psc", bufs=4,
                                             space="PSUM"))

        def mm(out, lhsT, rhs, start=True, stop=True):
            nc.tensor.matmul(out[:] if hasattr(out, "tile") else out,
                             lhsT=lhsT, rhs=rhs, start=start,
                             stop=stop, skip_group_check=True)

        for t in range(NTILES):
            sl = slice(t * NT, (t + 1) * NT)

            msgT_t = dense.tile([M, NT], bf16, tag="msgT")
            nc.sync.dma_start(msgT_t[:], aps["msgTc"][:, sl])
            ST_t = dense.tile([M, NT], bf16, tag="ST")
            nc.sync.dma_start(ST_t[:], aps["STc"][:, sl])
            xdT_t = dense.tile([NR, NT], bf16, tag="xdT")
            nc.sync.dma_start(xdT_t[:], aps["xdTc"][:, sl])
            # per-tile [1, 9*NT]: 8 blocks of a_b then the count row
            ac_t = dense.tile([1, (BD + 1) * NT], bf16, tag="ac")
            nc.sync.dma_start(
                ac_t[:],
                aps["acc"][:, t * (BD + 1) * NT:(t + 1) * (BD + 1) * NT])

            # d = x_dist @ Wd
            ps_d = psc.tile([H, NT], f32, tag="ps")
            mm(ps_d[:], W("Wd"), xdT_t[:])
            d_sb = dense.tile([H, NT], f32, tag="d")
            nc.scalar.copy(d_sb[:], ps_d[:])

            # u = (S@Ws + c*bs) * d
            ps_u = psc.tile([H, NT], f32, tag="ps")
            mm(ps_u[:], W("Ws"), ST_t[:], start=True, stop=False)
            mm(ps_u[:], W("bs_row"), ac_t[:, BD * NT:(BD + 1) * NT],
               start=False, stop=True)
            u_sb = dense.tile([H, NT], f32, tag="u")
            nc.vector.tensor_tensor(out=u_sb[:], in0=ps_u[:], in1=d_sb[:],
                                    op=mult)

            # x0 = agg + msg@Wt + bt    (accumulated in one PSUM tile)
            ps_x0 = pacc.tile([H, NT], f32, tag="pacc")
            mm(ps_x0[:], W("Wt"), msgT_t[:], start=True, stop=False)
            mm(ps_x0[:], W("bt_row"), ones_row[:], start=False, stop=False)
            for b in range(BD):
                bsl = slice(b * 128, (b + 1) * 128)
                ps_a = psc.tile([H, NT], f32, tag="ps")
                mm(ps_a[:], ones_col[:], ac_t[:, b * NT:(b + 1) * NT])
                z_sb = dense.tile([H, NT], bf16, tag="z")
                nc.vector.tensor_tensor(out=z_sb[:], in0=ps_a[:],
                                        in1=u_sb[:], op=mult)
                mm(ps_x0[:], W("WbT")[:, bsl], z_sb[:], start=False,
                   stop=(b == BD - 1))
            x0_sb = dense.tile([H, NT], bf16, tag="x0")
            nc.scalar.copy(x0_sb[:], ps_x0[:])

            # residual block (H)
            ps_h = psc.tile([H, NT], f32, tag="ps")
            mm(ps_h[:], W("rb_w1"), x0_sb[:])
            h1_sb = dense.tile([H, NT], bf16, tag="h1")
            nc.scalar.activation(h1_sb[:], ps_h[:], Silu, bias=B("rb_b1"))
            ps_h2 = psc.tile([H, NT], f32, tag="ps")
            mm(ps_h2[:], W("rb_w2"), h1_sb[:])
            h2_sb = dense.tile([H, NT], bf16, tag="h2")
            nc.scalar.activation(h2_sb[:], ps_h2[:], Silu, bias=B("rb_b2"))

            # skip: y = silu((x0+h2)@Wskip + bskip) + msg
            ps_y = pacc.tile([H, NT], f32, tag="pacc")
            mm(ps_y[:], W("Wskip"), x0_sb[:], start=True, stop=False)
            mm(ps_y[:], W("Wskip"), h2_sb[:], start=False, stop=True)
            ys_sb = dense.tile([M, NT], bf16, tag="ys")
            nc.scalar.activation(ys_sb[:], ps_y[:], Silu, bias=B("bskip"))
            y_sb = dense.tile([M, NT], bf16, tag="y")
            nc.vector.tensor_add(out=y_sb[:], in0=ys_sb[:], in1=msgT_t[:])

            # residual after 1
            ps_h = psc.tile([M, NT], f32, tag="ps")
            mm(ps_h[:], W("ra1_w1"), y_sb[:])
            h1p = dense.tile([M, NT], bf16, tag="h1")
            nc.scalar.activation(h1p[:], ps_h[:], Silu, bias=B("ra1_b1"))
            ps_h2 = psc.tile([M, NT], f32, tag="ps")
            mm(ps_h2[:], W("ra1_w2"), h1p[:])
            h2p = dense.tile([M, NT], bf16, tag="h2")
            nc.scalar.activation(h2p[:], ps_h2[:], Silu, bias=B("ra1_b2"))
            x2_sb = dense.tile([M, NT], bf16, tag="x2")
            nc.vector.tensor_add(out=x2_sb[:], in0=y_sb[:], in1=h2p[:])

            # residual after 2
            ps_h = psc.tile([M, NT], f32, tag="ps")
            mm(ps_h[:], W("ra2_w1"), x2_sb[:])
            h1q = dense.tile([M, NT], bf16, tag="h1")
            nc.scalar.activation(h1q[:], ps_h[:], Silu, bias=B("ra2_b1"))
            ps_h2 = psc.tile([M, NT], f32, tag="ps")
            mm(ps_h2[:], W("ra2_w2"), h1q[:])
            h2q = dense.tile([M, NT], bf16, tag="h2")
            nc.scalar.activation(h2q[:], ps_h2[:], Silu, bias=B("ra2_b2"))
            o_sb = dense.tile([M, NT], bf16, tag="o")
            nc.vector.tensor_add(out=o_sb[:], in0=x2_sb[:], in1=h2q[:])

            nc.sync.dma_start(aps["outT"][:, sl], o_sb[:])


def _run_custom(nc, concat_arrs, dev_in):
    """Thin PJRT runner: no zero-donation buffers (the kernel writes every
    output element), device_put already issued by the caller."""
    import jax
    from concourse import mybir
    from concourse.bass2jax import (_bass_exec_p, install_neuronx_cc_hook,
                                    partition_id_tensor)
    try:
        from jax import shard_map
    except ImportError:
        from jax.experimental.shard_map import shard_map
    from jax.sharding import Mesh, PartitionSpec

    install_neuronx_cc_hook()
    partition_name = (nc.partition_id_tensor.name
                      if nc.partition_id_tensor else None)
    in_names, out_names, out_avals = [], [], []
    for alloc in nc.m.functions[0].allocations:
        if not isinstance(alloc, mybir.MemoryLocationSet):
            continue
        name = alloc.memorylocations[0].name
        if alloc.kind == "ExternalInput":
            if name != partition_name:
                in_names.append(name)
        elif alloc.kind == "ExternalOutput":
            out_names.append(name)
            out_avals.append(jax.core.ShapedArray(
                tuple(alloc.tensor_shape), mybir.dt.np(alloc.dtype)))
    in_names_all = in_names + ([partition_name] if partition_name else [])

    def _body(*args):
        operands = list(args)
        if partition_name is not None:
            operands.append(partition_id_tensor())
        return tuple(_bass_exec_p.bind(
            *operands, out_avals=tuple(out_avals),
            in_names=tuple(in_names_all), out_names=tuple(out_names),
            lowering_input_output_aliases=(), sim_require_finite=True,
            sim_require_nnan=True, nc=nc))

    devices = jax.devices()[:P]
    mesh = Mesh(np.asarray(devices), ("core",))
    sharded = jax.jit(
        shard_map(_body, mesh=mesh,
                  in_specs=(PartitionSpec("core"),) * len(in_names),
                  out_specs=(PartitionSpec("core"),) * len(out_names),
                  check_rep=False),
        keep_unused=True)

    args = [dev_in[nm] for nm in in_names]
    compiled = sharded.lower(*args).compile()
    outs = compiled(*args)
    res = np.asarray(outs[0]).reshape(P, *out_avals[0].shape)
    return res


def kernel(**inputs):
    inputs = {k: np.asarray(v) for k, v in inputs.items()}
    bf16 = _bf16()
    x_dist = inputs["x_dist"].astype(np.float32)
    x_angle = inputs["x_angle"].astype(np.float32)
    msg = inputs["msg"].astype(np.float32)
    angle_index = inputs["angle_index"]

    import jax
    from jax.sharding import Mesh, NamedSharding, PartitionSpec

    devices = jax.devices()[:P]
    mesh = Mesh(np.asarray(devices), ("core",))
    sh = NamedSharding(mesh, PartitionSpec("core"))

    # ---- host prep, issuing async host->device copies as arrays complete --
    w = {k: np.asarray(inputs[k], np.float32) for k in (
        "Wd", "Wa", "Ws", "Wt", "Wb", "rb_w1", "rb_w2", "Wskip",
        "ra1_w1", "ra1_w2", "ra2_w1", "ra2_w2")}
    w["bs_row"] = inputs["bs"].reshape(1, H).astype(np.float32)
    w["bt_row"] = inputs["bt"].reshape(1, H).astype(np.float32)
    WbT = np.empty((H, BD * H), np.float32)
    for b in range(BD):
        WbT[:, b * 128:(b + 1) * 128] = w["Wb"][:, b, :].T
    w["WbT"] = WbT
    for name in _BIAS_NAMES:
        w[name] = np.asarray(inputs[name], np.float32)
    WB, BIA = _pack_weights(w)

    src = np.asarray(angle_index[0]).astype(np.int64)
    tgt = np.asarray(angle_index[1]).astype(np.int64)
    cnt = np.bincount(tgt, minlength=E).astype(np.float32)
    a = x_angle.reshape(E, NS * NR) @ w["Wa"]      # [E, BD]

    dev_in = {}
    concat = {}

    def stage(name, per_core_fn):
        arr = np.concatenate([per_core_fn(p) for p in range(P)], axis=0)
        concat[name] = arr
        dev_in[name] = jax.device_put(arr, sh)    # async

    def slab128(vals):   # [E, K] -> per-core [K, NSP] bf16
        def f(p):
            out = np.zeros((vals.shape[1], NSP), bf16)
            out[:, :ES] = vals[p * ES:(p + 1) * ES].T
            return out
        return f

    stage("WB", lambda p: WB)
    stage("BIA", lambda p: BIA)
    stage("msgTc", slab128(msg))
    stage("xdTc", slab128(x_dist))

    def acc_core(p):
        lo = p * ES
        block = np.zeros((BD + 1, NSP), np.float32)
        block[:BD, :ES] = a[lo:lo + ES].T
        block[BD, :ES] = cnt[lo:lo + ES]
        # tile-major [NTILES][BD+1][NT] on one partition row
        return np.ascontiguousarray(
            block.reshape(BD + 1, NTILES, NT).transpose(1, 0, 2)
        ).reshape(1, NTILES * (BD + 1) * NT).astype(bf16)

    stage("acc", acc_core)

    # segment-sum last (the most host work) so earlier copies stream behind it
    from scipy import sparse
    A = src.shape[0]
    C = sparse.csr_matrix((np.ones(A, np.float32), (tgt, src)), shape=(E, E))
    S = C @ msg                                    # [E, M]
    stage("STc", slab128(S))

    # ---- build + compile while transfers stream --------------------------
    import concourse.tile as tile
    from concourse import bacc, mybir

    nc = bacc.Bacc("TRN2", target_bir_lowering=False, debug=False,
                   enable_asserts=False, num_devices=P)
    aps = {}
    for name, arr in [("WB", WB), ("BIA", BIA),
                      ("msgTc", np.zeros((M, NSP), bf16)),
                      ("xdTc", np.zeros((NR, NSP), bf16)),
                      ("acc", np.zeros((1, NTILES * (BD + 1) * NT), bf16)),
                      ("STc", np.zeros((M, NSP), bf16))]:
        aps[name] = nc.dram_tensor(
            name, arr.shape, mybir.dt.from_np(arr.dtype),
            kind="ExternalInput").ap()
    aps["outT"] = nc.dram_tensor(
        "outT", (M, NSP), mybir.dt.bfloat16, kind="ExternalOutput").ap()

    with tile.TileContext(nc) as tc:
        _build(nc, tc, aps)
    nc.compile()

    try:
        res = _run_custom(nc, concat, dev_in)
    except Exception:
        # fallback: stock SPMD runner on the same module
        from concourse import bass_utils
        in_maps = []
        for p in range(P):
            in_maps.append({name: concat[name].reshape(
                P, *(concat[name].shape[0] // P, *concat[name].shape[1:]))[p]
                for name in concat})
        r = bass_utils.run_bass_kernel_spmd(
            nc, in_maps, core_ids=list(range(P)))
        res = np.stack([r.results[p]["outT"] for p in range(P)])

    out = np.empty((E, M), np.float32)
    for p in range(P):
        out[p * ES:(p + 1) * ES] = res[p][:, :ES].T.astype(np.float32)
    return out


# revision 13
# speedup vs baseline: 13.1603x; 13.1603x over previous
"""Trainium2 Bass kernel for nn_InteractionBlock (gnn_message_passing).

Algebraic transformation: per angle alpha with (s, t) = (src, tgt):
    sm[alpha] = (msg[s] @ Ws + bs) * d[t]
    out[alpha] = sum_b a[t, b] * (Wb[:, b, :] @ sm[alpha])
    agg[t] = sum_{alpha: tgt=t} out[alpha]
Everything except msg[s] depends only on t, so with
    S[t] = sum_{alpha: tgt=t} msg[s(alpha)]   and  c[t] = |{alpha: tgt=t}|
    agg[t] = sum_b a[t,b] * (Wb[:,b,:] @ ((S[t] @ Ws + c[t]*bs) * d[t]))
The A=600K gather+einsum+scatter collapses to an E-sized dense pipeline
after a segment-sum of raw msg rows.

The wall-clock budget is dominated by the axon host->device link, so
everything is tuned for bytes-on-the-wire: msg ships exactly once as bf16
row-major shards; an on-device AllGather replicates it, indirect-DMA
gathers + PE transpose-accumulate build the segment sums, and local PE
transposes build the feature-major msg slab for the dense per-edge
pipeline. a = x_angle @ Wa ([E, 8]) is computed on the host (tiny), all
operands are packed into a handful of arrays, host->device copies are
issued asynchronously before the XLA/NEFF compile so they overlap, and no
zero-donation buffers are shipped (the kernel writes every output
element). Edges are sharded 8 ways; per-core targets are processed in
count-descending order so the slot table stays dense.
"""

import os
import sys

import numpy as np

sys.path.insert(0, "/opt/trn_rl_repo")

E = 100000
NR = 6
NS = 7
H = 128
BD = 8
M = 128
P = 8           # cores
ES = E // P     # 12500 edges per core
NT = 512        # dense-phase column tile
NSP = 12800     # padded edges per core (25 * 512)
NTILES = NSP // NT  # 25
NCH = NSP // 128    # 100 slot chunks of 128 targets
ZROW = E            # index of the all-zero row in the gathered msg table

# packed bf16 weight slab: name -> (col offset, rows, cols)
_WOFF = {}
_c = 0
for _n, _r, _k in [("Wd", NR, H), ("Ws", M, H), ("bs_row", 1, H),
                   ("Wt", M, H), ("bt_row", 1, H), ("WbT", H, BD * H),
                   ("rb_w1", H, H), ("rb_w2", H, H), ("Wskip", H, M),
                   ("ra1_w1", M, M), ("ra1_w2", M, M),
                   ("ra2_w1", M, M), ("ra2_w2", M, M)]:
    _WOFF[_n] = (_c, _r, _k)
    _c += _k
WCOLS = _c  # 2560
_BIAS_NAMES = ["rb_b1", "rb_b2", "bskip", "ra1_b1", "ra1_b2",
               "ra2_b1", "ra2_b2"]


def _bf16():
    import ml_dtypes
    return np.dtype(ml_dtypes.bfloat16)


def _pack_weights(w):
    bf16 = _bf16()
    WB = np.zeros((128, WCOLS), bf16)
    for name, (c0, rows, cols) in _WOFF.items():
        WB[:rows, c0:c0 + cols] = w[name]
    BIA = np.zeros((128, len(_BIAS_NAMES)), np.float32)
    for i, name in enumerate(_BIAS_NAMES):
        BIA[:, i] = w[name]
    return WB, BIA


def _build_slots(src, tgt):
    """Per-core count-descending permutations and the common slot table
    geometry. Returns (perms, ncols [NCH], slots list of [128, tot_cols])."""
    cnt = np.bincount(tgt, minlength=E).astype(np.int64)
    order = np.argsort(tgt, kind="stable")
    srcs_by_tgt = src[order]
    starts = np.zeros(E + 1, np.int64)
    np.cumsum(cnt, out=starts[1:])

    perms = []
    core_cols = []
    for p in range(P):
        lo = p * ES
        cp = cnt[lo:lo + ES]
        perm = np.argsort(-cp, kind="stable")
        perms.append(perm)
        cps_pad = np.zeros(NSP, np.int64)
        cps_pad[:ES] = cp[perm]
        core_cols.append(cps_pad.reshape(NCH, 128).max(axis=1))
    ncols = np.maximum(np.maximum.reduce(core_cols), 1)
    tot_cols = int(ncols.sum())

    # global permuted position of each edge (for slot values)
    pos = np.empty(E, np.int64)
    for p in range(P):
        pos[p * ES + perms[p]] = p * ES + np.arange(ES)

    slots_list = []
    maxc = int(ncols.max())
    for p in range(P):
        lo = p * ES
        perm = perms[p]
        gperm = perm + lo
        cps = cnt[gperm]
        slot = np.full((NSP, maxc), ZROW, np.int32)
        reps = cps
        ii = np.repeat(np.arange(ES), reps)
        jj = (np.arange(len(ii)) -
              np.repeat(np.concatenate(([0], np.cumsum(reps)[:-1])), reps))
        gt = np.repeat(gperm, reps)
        sstart = starts[gt] + jj
        slot[ii, jj] = pos[srcs_by_tgt[sstart]]
        cols = np.concatenate(
            [slot[c * 128:(c + 1) * 128, :ncols[c]] for c in range(NCH)],
            axis=1)
        slots_list.append(np.ascontiguousarray(cols))
    return perms, ncols, tot_cols, cnt, slots_list


def _build(nc, tc, aps, ncols):
    """Emit the kernel IR: AllGather msg, on-device segment-sum + local
    transpose, then the dense per-edge pipeline (feature-major, bf16)."""
    from contextlib import ExitStack

    import concourse.bass as bass
    from concourse import mybir
    from concourse.bass import IndirectOffsetOnAxis
    from concourse.masks import make_identity

    f32 = mybir.dt.float32
    bf16 = mybir.dt.bfloat16
    i32 = mybir.dt.int32
    Silu = mybir.ActivationFunctionType.Silu
    mult = mybir.AluOpType.mult
    tot_cols = int(ncols.sum())

    with ExitStack() as ctx:
        wpool = ctx.enter_context(tc.tile_pool(name="w", bufs=1))
        dramp = ctx.enter_context(tc.tile_pool(name="dram", bufs=1,
                                               space="DRAM"))
        slab = ctx.enter_context(tc.tile_pool(name="slab", bufs=1))

        # ---- msg AllGather: ship once, replicate on NeuronLink ----------
        inb = dramp.tile([ES, M], bf16, tag="inb")
        nc.gpsimd.dma_start(inb[:], aps["msgR"][:])
        outb = dramp.tile([E + 1, M], bf16, tag="outb")
        nc.gpsimd.collective_compute(
            "AllGather", mybir.AluOpType.bypass,
            replica_groups=[list(range(P))],
            ins=[inb[:].opt()], outs=[outb[0:E, :].opt()])

        wslab = wpool.tile([128, WCOLS], bf16, tag="WB")
        nc.sync.dma_start(wslab[:], aps["WB"][:])
        bias = wpool.tile([128, len(_BIAS_NAMES)], f32, tag="BIA")
        nc.sync.dma_start(bias[:], aps["BIA"][:])
        slots_sb = wpool.tile([128, tot_cols], i32, tag="slots")
        nc.sync.dma_start(slots_sb[:], aps["slots"][:])

        def W(name):
            c0, rows, cols = _WOFF[name]
            return wslab[0:rows, c0:c0 + cols]

        def B(name):
            return bias[:, _BIAS_NAMES.index(name):_BIAS_NAMES.index(name) + 1]

        ones_row = wpool.tile([1, NT], bf16, tag="ones")
        nc.gpsimd.memset(ones_row[:], 1.0)
        ones_col = wpool.tile([1, 128], bf16, tag="onesc")
        nc.gpsimd.memset(ones_col[:], 1.0)
        identb = wpool.tile([128, 128], bf16, tag="identb")
        make_identity(nc, identb[:])
        zrow = wpool.tile([1, M], bf16, tag="zrow")
        nc.gpsimd.memset(zrow[:], 0.0)
        nc.sync.dma_start(outb[E:E + 1, :], zrow[:])

        # feature-major resident slabs, built on device
        msgT = slab.tile([M, NSP], bf16, tag="msgT")
        ST = slab.tile([M, NSP], bf16, tag="ST")

        # ---- local transpose: msgT[:, :ES] = msgR^T ---------------------
        with tc.tile_pool(name="tp", bufs=4) as tpool, \
             tc.tile_pool(name="ptp", bufs=4, space="PSUM") as ptpool:
            zpad = tpool.tile([128, NSP - ES], bf16, tag="zpad")
            nc.gpsimd.memset(zpad[:], 0.0)
            nc.scalar.copy(msgT[:, ES:NSP], zpad[:])
            nc.scalar.copy(ST[:, ES:NSP], zpad[:])
            for c in range((ES + 127) // 128):
                r0 = c * 128
                rows = min(128, ES - r0)
                g = tpool.tile([128, M], bf16, tag="g")
                nc.sync.dma_start(g[0:rows, :], aps["msgR"][r0:r0 + rows, :])
                ps = ptpool.tile([128, 128], f32, tag="pt")
                nc.tensor.matmul(ps[:, 0:rows], lhsT=g[0:rows, :],
                                 rhs=identb[0:rows, 0:rows],
                                 start=True, stop=True,
                                 skip_group_check=True)
                nc.scalar.copy(msgT[:, r0:r0 + rows], ps[:, 0:rows])

        # ---- on-device segment sum via slot gathers ---------------------
        with tc.tile_pool(name="gth", bufs=12) as gpool, \
             tc.tile_pool(name="pgt", bufs=4, space="PSUM") as pgpool:
            col = 0
            for c in range(NCH):
                nj = int(ncols[c])
                ps = pgpool.tile([128, 128], f32, tag="pg")
                for j in range(nj):
                    g = gpool.tile([128, M], bf16, tag="g")
                    nc.gpsimd.indirect_dma_start(
                        out=g[:], out_offset=None,
                        in_=outb[:],
                        in_offset=IndirectOffsetOnAxis(
                            ap=slots_sb[:, col + j:col + j + 1], axis=0),
                    )
                    nc.tensor.matmul(
                        ps[:], lhsT=g[:], rhs=identb[:],
                        start=(j == 0), stop=(j == nj - 1),
                        skip_group_check=True)
                nc.scalar.copy(ST[:, c * 128:(c + 1) * 128], ps[:])
                col += nj

        # ---- dense per-edge pipeline ------------------------------------
        dense = ctx.enter_context(tc.tile_pool(name="dn", bufs=3))
        pacc = ctx.enter_context(tc.tile_pool(name="pacc", bufs=2,
                                              space="PSUM"))
        psc = ctx.enter_context(tc.tile_pool(name="psc", bufs=4,
                                             space="PSUM"))

        def mm(out, lhsT, rhs, start=True, stop=True):
            nc.tensor.matmul(out, lhsT=lhsT, rhs=rhs, start=start,
                             stop=stop, skip_group_check=True)

        for t in range(NTILES):
            sl = slice(t * NT, (t + 1) * NT)

            xdT_t = dense.tile([NR, NT], bf16, tag="xdT")
            nc.sync.dma_start(xdT_t[:], aps["xdTc"][:, sl])
            ac_t = dense.tile([1, (BD + 1) * NT], bf16, tag="ac")
            nc.sync.dma_start(
                ac_t[:],
                aps["acc"][:, t * (BD + 1) * NT:(t + 1) * (BD + 1) * NT])

            # d = x_dist @ Wd
            ps_d = psc.tile([H, NT], f32, tag="ps")
            mm(ps_d[:], W("Wd"), xdT_t[:])
            d_sb = dense.tile([H, NT], f32, tag="d")
            nc.scalar.copy(d_sb[:], ps_d[:])

            # u = (S@Ws + c*bs) * d
            ps_u = psc.tile([H, NT], f32, tag="ps")
            mm(ps_u[:], W("Ws"), ST[:, sl], start=True, stop=False)
            mm(ps_u[:], W("bs_row"), ac_t[:, BD * NT:(BD + 1) * NT],
               start=False, stop=True)
            u_sb = dense.tile([H, NT], f32, tag="u")
            nc.vector.tensor_tensor(out=u_sb[:], in0=ps_u[:], in1=d_sb[:],
                                    op=mult)

            # x0 = agg + msg@Wt + bt    (accumulated in one PSUM tile)
            ps_x0 = pacc.tile([H, NT], f32, tag="pacc")
            mm(ps_x0[:], W("Wt"), msgT[:, sl], start=True, stop=False)
            mm(ps_x0[:], W("bt_row"), ones_row[:], start=False, stop=False)
            for b in range(BD):
                bsl = slice(b * 128, (b + 1) * 128)
                ps_a = psc.tile([H, NT], f32, tag="ps")
                mm(ps_a[:], ones_col[:], ac_t[:, b * NT:(b + 1) * NT])
                z_sb = dense.tile([H, NT], bf16, tag="z")
                nc.vector.tensor_tensor(out=z_sb[:], in0=ps_a[:],
                                        in1=u_sb[:], op=mult)
                mm(ps_x0[:], W("WbT")[:, bsl], z_sb[:], start=False,
                   stop=(b == BD - 1))
            x0_sb = dense.tile([H, NT], bf16, tag="x0")
            nc.scalar.copy(x0_sb[:], ps_x0[:])

            # residual block (H)
            ps_h = psc.tile([H, NT], f32, tag="ps")
            mm(ps_h[:], W("rb_w1"), x0_sb[:])
            h1_sb = dense.tile([H, NT], bf16, tag="h1")
            nc.scalar.activation(h1_sb[:], ps_h[:], Silu, bias=B("rb_b1"))
            ps_h2 = psc.tile([H, NT], f32, tag="ps")
            mm(ps_h2[:], W("rb_w2"), h1_sb[:])
            h2_sb = dense.tile([H, NT], bf16, tag="h2")
            nc.scalar.activation(h2_sb[:], ps_h2[:], Silu, bias=B("rb_b2"))

            # skip: y = silu((x0+h2)@Wskip + bskip) + msg
            ps_y = pacc.tile([H, NT], f32, tag="pacc")
            mm(ps_y[:], W("Wskip"), x0_sb[:], start=True, stop=False)
            mm(ps_y[:], W("Wskip"), h2_sb[:], start=False, stop=True)
            ys_sb = dense.tile([M, NT], bf16, tag="ys")
            nc.scalar.activation(ys_sb[:], ps_y[:], Silu, bias=B("bskip"))
            y_sb = dense.tile([M, NT], bf16, tag="y")
            nc.vector.tensor_add(out=y_sb[:], in0=ys_sb[:], in1=msgT[:, sl])

            # residual after 1
            ps_h = psc.tile([M, NT], f32, tag="ps")
            mm(ps_h[:], W("ra1_w1"), y_sb[:])
            h1p = dense.tile([M, NT], bf16, tag="h1")
            nc.scalar.activation(h1p[:], ps_h[:], Silu, bias=B("ra1_b1"))
            ps_h2 = psc.tile([M, NT], f32, tag="ps")
            mm(ps_h2[:], W("ra1_w2"), h1p[:])
            h2p = dense.tile([M, NT], bf16, tag="h2")
            nc.scalar.activation(h2p[:], ps_h2[:], Silu, bias=B("ra1_b2"))
            x2_sb = dense.tile([M, NT], bf16, tag="x2")
            nc.vector.tensor_add(out=x2_sb[:], in0=y_sb[:], in1=h2p[:])

            # residual after 2
            ps_h = psc.tile([M, NT], f32, tag="ps")
            mm(ps_h[:], W("ra2_w1"), x2_sb[:])
            h1q = dense.tile([M, NT], bf16, tag="h1")
            nc.scalar.activation(h1q[:], ps_h[:], Silu, bias=B("ra2_b1"))
            ps_h2 = psc.tile([M, NT], f32, tag="ps")
            mm(ps_h2[:], W("ra2_w2"), h1q[:])
            h2q = dense.tile([M, NT], bf16, tag="h2")
            nc.scalar.activation(h2q[:], ps_h2[:], Silu, bias=B("ra2_b2"))
            o_sb = dense.tile([M, NT], bf16, tag="o")
            nc.vector.tensor_add(out=o_sb[:], in0=x2_sb[:], in1=h2q[:])

            nc.sync.dma_start(aps["outT"][:, sl], o_sb[:])


def _run_custom(nc, dev_in):
    """Thin PJRT runner: no zero-donation buffers (the kernel writes every
    output element); device_put already issued by the caller."""
    import jax
    from concourse import mybir
    from concourse.bass2jax import (_bass_exec_p, install_neuronx_cc_hook,
                                    partition_id_tensor)
    try:
        from jax import shard_map
    except ImportError:
        from jax.experimental.shard_map import shard_map
    from jax.sharding import Mesh, PartitionSpec

    install_neuronx_cc_hook()
    partition_name = (nc.partition_id_tensor.name
                      if nc.partition_id_tensor else None)
    in_names, out_names, out_avals = [], [], []
    for alloc in nc.m.functions[0].allocations:
        if not isinstance(alloc, mybir.MemoryLocationSet):
            continue
        name = alloc.memorylocations[0].name
        if alloc.kind == "ExternalInput":
            if name != partition_name:
                in_names.append(name)
        elif alloc.kind == "ExternalOutput":
            out_names.append(name)
            out_avals.append(jax.core.ShapedArray(
                tuple(alloc.tensor_shape), mybir.dt.np(alloc.dtype)))
    in_names_all = in_names + ([partition_name] if partition_name else [])

    def _body(*args):
        operands = list(args)
        if partition_name is not None:
            operands.append(partition_id_tensor())
        return tuple(_bass_exec_p.bind(
            *operands, out_avals=tuple(out_avals),
            in_names=tuple(in_names_all), out_names=tuple(out_names),
            lowering_input_output_aliases=(), sim_require_finite=True,
            sim_require_nnan=True, nc=nc))

    devices = jax.devices()[:P]
    mesh = Mesh(np.asarray(devices), ("core",))
    sharded = jax.jit(
        shard_map(_body, mesh=mesh,
                  in_specs=(PartitionSpec("core"),) * len(in_names),
                  out_specs=(PartitionSpec("core"),) * len(out_names),
                  check_rep=False),
        keep_unused=True)

    args = [dev_in[nm] for nm in in_names]
    compiled = sharded.lower(*args).compile()
    outs = compiled(*args)
    return np.asarray(outs[0]).reshape(P, *out_avals[0].shape)


def kernel(**inputs):
    inputs = {k: np.asarray(v) for k, v in inputs.items()}
    bf16 = _bf16()
    x_dist = inputs["x_dist"].astype(np.float32)
    x_angle = inputs["x_angle"].astype(np.float32)
    msg = inputs["msg"].astype(np.float32)
    angle_index = inputs["angle_index"]

    import jax
    from jax.sharding import Mesh, NamedSharding, PartitionSpec

    devices = jax.devices()[:P]
    mesh = Mesh(np.asarray(devices), ("core",))
    sh = NamedSharding(mesh, PartitionSpec("core"))

    dev_in = {}
    concat = {}

    def stage(name, arrs):
        arr = np.concatenate(arrs, axis=0)
        concat[name] = arr
        dev_in[name] = jax.device_put(arr, sh)    # async

    # ---- host prep, streaming copies as soon as each array is ready ----
    w = {k: np.asarray(inputs[k], np.float32) for k in (
        "Wd", "Wa", "Ws", "Wt", "Wb", "rb_w1", "rb_w2", "Wskip",
        "ra1_w1", "ra1_w2", "ra2_w1", "ra2_w2")}
    w["bs_row"] = inputs["bs"].reshape(1, H).astype(np.float32)
    w["bt_row"] = inputs["bt"].reshape(1, H).astype(np.float32)
    WbT = np.empty((H, BD * H), np.float32)
    for b in range(BD):
        WbT[:, b * 128:(b + 1) * 128] = w["Wb"][:, b, :].T
    w["WbT"] = WbT
    for name in _BIAS_NAMES:
        w[name] = np.asarray(inputs[name], np.float32)
    WB, BIA = _pack_weights(w)
    stage("WB", [WB] * P)
    stage("BIA", [BIA] * P)

    src = np.asarray(angle_index[0]).astype(np.int64)
    tgt = np.asarray(angle_index[1]).astype(np.int64)
    perms, ncols, tot_cols, cnt, slots_list = _build_slots(src, tgt)

    stage("msgR", [msg[p * ES:(p + 1) * ES][perms[p]].astype(bf16)
                   for p in range(P)])
    stage("slots", slots_list)

    a = x_angle.reshape(E, NS * NR) @ w["Wa"]      # [E, BD]

    def slab(vals, rows):   # [E, K] -> per-core [K, NSP] bf16, permuted
        outs = []
        for p in range(P):
            o = np.zeros((rows, NSP), bf16)
            o[:, :ES] = vals[p * ES:(p + 1) * ES][perms[p]].T
            outs.append(o)
        return outs

    stage("xdTc", slab(x_dist, NR))

    acc_list = []
    cntf = cnt.astype(np.float32)
    for p in range(P):
        lo = p * ES
        block = np.zeros((BD + 1, NSP), np.float32)
        block[:BD, :ES] = a[lo:lo + ES][perms[p]].T
        block[BD, :ES] = cntf[lo:lo + ES][perms[p]]
        acc_list.append(np.ascontiguousarray(
            block.reshape(BD + 1, NTILES, NT).transpose(1, 0, 2)
        ).reshape(1, NTILES * (BD + 1) * NT).astype(bf16))
    stage("acc", acc_list)

    # ---- build + compile while transfers stream ------------------------
    import concourse.tile as tile
    from concourse import bacc, mybir

    nc = bacc.Bacc("TRN2", target_bir_lowering=False, debug=False,
                   enable_asserts=False, num_devices=P)
    aps = {}
    for name, shape, dt in [
            ("WB", (128, WCOLS), "bf16"), ("BIA", (128, 7), "f32"),
            ("msgR", (ES, M), "bf16"), ("slots", (128, tot_cols), "i32"),
            ("xdTc", (NR, NSP), "bf16"),
            ("acc", (1, NTILES * (BD + 1) * NT), "bf16")]:
        mdt = {"bf16": mybir.dt.bfloat16, "f32": mybir.dt.float32,
               "i32": mybir.dt.int32}[dt]
        aps[name] = nc.dram_tensor(name, shape, mdt,
                                   kind="ExternalInput").ap()
    aps["outT"] = nc.dram_tensor(
        "outT", (M, NSP), mybir.dt.bfloat16, kind="ExternalOutput").ap()

    with tile.TileContext(nc) as tc:
        _build(nc, tc, aps, ncols)
    nc.compile()

    try:
        res = _run_custom(nc, dev_in)
    except Exception:
        from concourse import bass_utils
        in_maps = []
        for p in range(P):
            in_maps.append({name: concat[name].reshape(
                (P, concat[name].shape[0] // P) + concat[name].shape[1:])[p]
                for name in concat})
        r = bass_utils.run_bass_kernel_spmd(
            nc, in_maps, core_ids=list(range(P)))
        res = np.stack([r.results[p]["outT"] for p in range(P)])

    out = np.empty((E, M), np.float32)
    for p in range(P):
        out[p * ES + perms[p]] = res[p][:, :ES].T.astype(np.float32)
    return out
